# revision 1
# baseline (speedup 1.0000x reference)
"""Trainium2 Bass kernel for nn_Attention_linearCombination.

out = sum_i softmax_i(tanh(x_i @ W_att_i + b_att_i) @ v) * (x_i @ W_tr_i + b_tr_i)

Sharding: data-parallel over the batch dim (16384 -> 8 cores x 2048 rows);
weights replicated. Per core, 16 tiles of 128 rows:
  - x tiles cast-loaded fp32->bf16 by SWDGE DMA, transposed by the xbar
    DMA-transpose into [128, 8, 128] (k-major) for the PE.
  - PE: per branch, 8 accumulating bf16 matmuls for x@W_att (N=256) and
    x@W_tr (N=512) with the transposed x chunk as the stationary operand;
    same-PSUM-bank matmul groups are kept contiguous (bank switches cost a
    pipeline re-setup). b_tr is added via a K=1 ones-row matmul; b_att is
    added on DVE (broadcast once at setup) to keep PE work minimal.
  - tanh via sigmoid identity (tanh(a) = 2*sigmoid(2a) - 1; Tanh ACT table
    crashes the device on this runtime), logits l_i = 2*(sigmoid(2a) @ v) - sum(v).
  - softmax via sigma-ratio (e^l = sig(l)/(1-sig(l))) to stay on the
    sigmoid ACT table set (exp lives in a different set -> reload thrash;
    tensor_tensor_reduce also crashes the device -> separate mul+reduce).
  - combine: unscaled PSUM evacuation on ACT, then per-partition-scaled
    copies and two DVE adds; loads prefetch 2 tiles ahead (SWDGE/Pool ring),
    transposes 1 tile ahead (SP HWDGE ring), stores on the Pool ring so no
    HWDGE wait ever blocks the transpose sequencer.
"""
import numpy as np

import concourse.bass as bass
import concourse.bacc as bacc
import concourse.mybir as mybir
import concourse.tile as tile
from concourse.bass_utils import run_bass_kernel_spmd

F32 = mybir.dt.float32
BF16 = mybir.dt.bfloat16
AF = mybir.ActivationFunctionType
OP = mybir.AluOpType

B = 16384
D = 1024
INT = 256
OUT = 512
NB = 3
NCORES = 8
B_LOC = B // NCORES
KC = D // 128
N_TILES = B_LOC // 128

_CACHE = {}


def _build_nc(repeat=1, loop_repeat=1):
    nc = bacc.Bacc(None, target_bir_lowering=False, num_swdge_queues=2)
    xs = [nc.dram_tensor(f"x{i+1}", [B_LOC, D], F32, kind="ExternalInput") for i in range(NB)]
    Was = [nc.dram_tensor(f"W_att{i+1}", [D, INT], F32, kind="ExternalInput") for i in range(NB)]
    bas = [nc.dram_tensor(f"b_att{i+1}", [1, INT], F32, kind="ExternalInput") for i in range(NB)]
    Wts = [nc.dram_tensor(f"W_tr{i+1}", [D, OUT], F32, kind="ExternalInput") for i in range(NB)]
    bts = [nc.dram_tensor(f"b_tr{i+1}", [1, OUT], F32, kind="ExternalInput") for i in range(NB)]
    v = nc.dram_tensor("v", [INT, 1], F32, kind="ExternalInput")
    out = nc.dram_tensor("out", [B_LOC, OUT], F32, kind="ExternalOutput")

    with tile.TileContext(nc) as tc:
        with (
            tc.tile_pool(name="wpool", bufs=1) as wpool,
            tc.tile_pool(name="work", bufs=4) as pool,
            tc.tile_pool(name="xpool", bufs=6) as xpool,
            tc.tile_pool(name="psum", bufs=4, space="PSUM") as psum,
            tc.tile_pool(name="ptr", bufs=4, space="PSUM") as ptrpool,
        ):
            # ---- one-time setup: weights to SBUF as bf16, k-chunked ----
            Wa_sb, Wt_sb, ba_sb, bt_sb = [], [], [], []
            for i in range(NB):
                wa = wpool.tile([128, KC, INT], BF16, tag=f"wa{i}")
                nc.gpsimd.dma_start(out=wa[:], in_=Was[i].rearrange("(c p) n -> p c n", p=128))
                Wa_sb.append(wa)
                wt = wpool.tile([128, KC, OUT], BF16, tag=f"wt{i}")
                nc.gpsimd.dma_start(out=wt[:], in_=Wts[i].rearrange("(c p) n -> p c n", p=128))
                Wt_sb.append(wt)
                bav = wpool.tile([1, INT], BF16, tag=f"ba{i}")
                nc.gpsimd.dma_start(out=bav[:], in_=bas[i][:])
                ba_sb.append(bav)
                btv = wpool.tile([1, OUT], BF16, tag=f"bt{i}")
                nc.gpsimd.dma_start(out=btv[:], in_=bts[i][:])
                bt_sb.append(btv)
            ones16 = wpool.tile([1, 128], BF16, tag="ones16")
            nc.vector.memset(ones16[:], 1.0)
            ones32 = wpool.tile([1, 128], F32, tag="ones32")
            nc.vector.memset(ones32[:], 1.0)

            # b_att broadcast to all partitions (K=1 matmul, one-time) so the
            # per-tile bias add runs on DVE instead of PE (PE is the bottleneck)
            ba_rep = []
            for i in range(NB):
                p_b = psum.tile([128, INT], F32, tag="att")
                nc.tensor.matmul(p_b[:], lhsT=ones16[:], rhs=ba_sb[i][:], start=True, stop=True)
                bar = wpool.tile([128, INT], F32, tag=f"barep{i}")
                nc.scalar.activation(bar[:], p_b[:], AF.Copy)
                ba_rep.append(bar)

            # v broadcast to all partitions via K=1 fp32 matmul
            v_row = wpool.tile([1, INT], F32, tag="vrow")
            nc.sync.dma_start(out=v_row[:], in_=v.rearrange("a b -> b a"))
            p_v = psum.tile([128, INT], F32, tag="att")
            nc.tensor.matmul(p_v[:], lhsT=ones32[:], rhs=v_row[:], start=True, stop=True)
            v2_rep = wpool.tile([128, INT], F32, tag="v2rep")
            nc.scalar.activation(v2_rep[:], p_v[:], AF.Copy, scale=2.0)
            vsum = wpool.tile([128, 1], F32, tag="vsum")
            nc.vector.reduce_sum(vsum[:], p_v[:], axis=mybir.AxisListType.X)

            # ---- main loop over 128-row tiles, software-pipelined ----
            # loads (SWDGE/Pool ring) run 2 tiles ahead, transposes (SP HWDGE
            # ring) 1 tile ahead, stores go on the Pool ring so they never
            # block the SP sequencer (HWDGE waits stall the issuing ring).
            import contextlib
            loop_cm = tc.For_i(0, loop_repeat, 1) if loop_repeat > 1 else contextlib.nullcontext()
            with loop_cm:
              xb_q, xT_q = {}, {}

              def issue_loads(t):
                  for i in range(NB):
                      xb = xpool.tile([128, D], BF16, tag=f"xb{i}")
                      nc.gpsimd.dma_start(out=xb[:], in_=xs[i][t * 128:(t + 1) * 128, :])
                      xb_q[(t, i)] = xb

              def issue_transposes(t):
                  for i in range(NB):
                      xT = xpool.tile([128, KC, 128], BF16, tag=f"xT{i}")
                      nc.sync.dma_start(out=xT[:], in_=xb_q.pop((t, i))[:], transpose=True)
                      xT_q[(t, i)] = xT

              tiles = [tt for _ in range(repeat) for tt in range(N_TILES)]
              issue_loads(tiles[0])
              if len(tiles) > 1:
                  issue_loads(tiles[1])
              issue_transposes(tiles[0])
              for ti, t in enumerate(tiles):
                if ti + 2 < len(tiles):
                    issue_loads(tiles[ti + 2])
                if ti + 1 < len(tiles):
                    issue_transposes(tiles[ti + 1])
                l3 = pool.tile([128, 4], F32, tag="l3")
                xTs = [xT_q.pop((t, i)) for i in range(NB)]
                # same-PSUM-bank matmuls stay contiguous: all att groups first,
                # then all tr groups (bank switches cost a pipeline re-setup)
                p_atts = []
                for i in range(NB):
                    p_att = psum.tile([128, INT], F32, tag="att")
                    for c in range(KC):
                        nc.tensor.matmul(p_att[:], lhsT=xTs[i][:, c, :], rhs=Wa_sb[i][:, c, :],
                                         start=(c == 0), stop=(c == KC - 1))
                    p_atts.append(p_att)
                p_trs = []
                for i in range(NB):
                    p_tr = ptrpool.tile([128, OUT], F32, tag="tr")
                    for c in range(KC):
                        nc.tensor.matmul(p_tr[:], lhsT=xTs[i][:, c, :], rhs=Wt_sb[i][:, c, :],
                                         start=(c == 0), stop=False)
                    nc.tensor.matmul(p_tr[:], lhsT=ones16[:], rhs=bt_sb[i][:], start=False, stop=True)
                    p_trs.append(p_tr)
                ptrs = []
                for i in range(NB):
                    # evacuate PSUM immediately (unscaled) so the bank frees
                    # without waiting for the softmax chain
                    traw = pool.tile([128, OUT], F32, tag=f"traw{i}")
                    nc.scalar.activation(traw[:], p_trs[i][:], AF.Copy)
                    ptrs.append(traw)

                    ab = pool.tile([128, INT], F32, tag="ab")
                    nc.vector.tensor_add(ab[:], p_atts[i][:], ba_rep[i][:])
                    sgh = pool.tile([128, INT], F32, tag="sgh")
                    nc.scalar.activation(sgh[:], ab[:], AF.Sigmoid, scale=2.0)
                    prod = pool.tile([128, INT], F32, tag="prod")
                    nc.vector.tensor_mul(prod[:], sgh[:], v2_rep[:])
                    raw = pool.tile([128, 1], F32, tag="raw")
                    nc.vector.reduce_sum(raw[:], prod[:], axis=mybir.AxisListType.X)
                    nc.vector.tensor_scalar(l3[:, i:i + 1], raw[:], vsum[:], None, OP.subtract)

                sg3 = pool.tile([128, 4], F32, tag="sg3")
                nc.scalar.activation(sg3[:, 0:NB], l3[:, 0:NB], AF.Sigmoid)
                u3 = pool.tile([128, 4], F32, tag="u3")
                nc.vector.tensor_scalar(u3[:, 0:NB], sg3[:, 0:NB], -1.0, 1.0, OP.mult, OP.add)
                w3 = pool.tile([128, 4], F32, tag="w3")
                nc.vector.reciprocal(w3[:, 0:NB], u3[:, 0:NB])
                r3 = pool.tile([128, 4], F32, tag="r3")
                nc.vector.tensor_mul(r3[:, 0:NB], sg3[:, 0:NB], w3[:, 0:NB])
                ssum = pool.tile([128, 1], F32, tag="ssum")
                nc.vector.reduce_sum(ssum[:], r3[:, 0:NB], axis=mybir.AxisListType.X)
                rs = pool.tile([128, 1], F32, tag="rs")
                nc.vector.reciprocal(rs[:], ssum[:])
                s3 = pool.tile([128, 4], F32, tag="s3")
                nc.vector.tensor_scalar_mul(s3[:, 0:NB], r3[:, 0:NB], rs[:])

                t0 = pool.tile([128, OUT], F32, tag="t0")
                t1 = pool.tile([128, OUT], F32, tag="t1")
                t2 = pool.tile([128, OUT], F32, tag="t2")
                for i, tt in enumerate([t0, t1, t2]):
                    nc.scalar.activation(tt[:], ptrs[i][:], AF.Copy, scale=s3[:, i:i + 1])
                a01 = pool.tile([128, OUT], F32, tag="a01")
                nc.vector.tensor_add(a01[:], t0[:], t1[:])
                acc = pool.tile([128, OUT], F32, tag="acc")
                nc.vector.tensor_add(acc[:], a01[:], t2[:])
                nc.gpsimd.dma_start(out=out[t * 128:(t + 1) * 128, :], in_=acc[:])
    nc.compile()
    return nc


LAST_RESULTS = None


def kernel(**inputs) -> np.ndarray:
    if "nc" not in _CACHE:
        _CACHE["nc"] = _build_nc()
    nc = _CACHE["nc"]

    shared = {}
    for i in range(NB):
        for k in (f"W_att{i+1}", f"b_att{i+1}", f"W_tr{i+1}", f"b_tr{i+1}"):
            shared[k] = np.ascontiguousarray(np.asarray(inputs[k], dtype=np.float32))
    shared["v"] = np.ascontiguousarray(np.asarray(inputs["v"], dtype=np.float32))

    in_maps = []
    for c in range(NCORES):
        m = dict(shared)
        for i in range(NB):
            m[f"x{i+1}"] = np.ascontiguousarray(
                np.asarray(inputs[f"x{i+1}"], dtype=np.float32)[c * B_LOC:(c + 1) * B_LOC]
            )
        in_maps.append(m)

    res = run_bass_kernel_spmd(nc, in_maps, core_ids=list(range(NCORES)))
    global LAST_RESULTS
    LAST_RESULTS = res
    return np.concatenate([r["out"] for r in res.results], axis=0)



# revision 2
# speedup vs baseline: 1.0160x; 1.0160x over previous
"""Trainium2 Bass kernel for nn_Attention_linearCombination — v7.

out = sum_i softmax_i(tanh(x_i @ W_att_i + b_att_i) @ v) * (x_i @ W_tr_i + b_tr_i)

Data-parallel over batch (16384 -> 8 cores x 2048 rows), weights replicated.
Per core, 16 tiles of 128 rows.

v7 key insight: SWDGE (gpsimd) DMAs and HWDGE DMA-transposes mutually
serialize (~12us per pair, HW-probed) — every cast-load blocked every
transpose in earlier versions. Steady state here uses NO SWDGE at all:
  - x loads: HWDGE plain fp32 on the ACT ring (~340GB/s), cast fp32->bf16
    on DVE (tensor_copy, ~1.9us/tile), per-branch [128,8,128] transposes on
    the SP ring (the 256KB shape runs ~300GB/s; batched shapes drop to 79).
  - weights: SWDGE cast-loads at setup only, completing before the first
    transpose is issued.
  - PE per branch: 8 accumulating bf16 matmuls + uniform K=128 bias matmul
    (ones128 @ b/128 broadcast) for att (N=256) and tr (N=512); uniform
    tile_size keeps att groups at the ~109ns/MM issue floor.
  - att post: ACT Sigmoid reads att PSUM (scale=2); one DVE
    scalar_tensor_tensor gives sgh*2v with accum_out = logit.
  - softmax: sg3 = Sigmoid(raw + bias=-sum(v)); sigma-ratio softmax, row-sum
    via stt accum_out. combine: 3 fused DVE stt ops reading tr PSUM.
  - stores on the ACT ring.
"""
import numpy as np

import concourse.bass as bass
import concourse.bacc as bacc
import concourse.mybir as mybir
import concourse.tile as tile
from concourse.bass_utils import run_bass_kernel_spmd

F32 = mybir.dt.float32
BF16 = mybir.dt.bfloat16
AF = mybir.ActivationFunctionType
OP = mybir.AluOpType

B = 16384
D = 1024
INT = 256
OUT = 512
NB = 3
NCORES = 8
B_LOC = B // NCORES
KC = D // 128
N_TILES = B_LOC // 128

_CACHE = {}


def _build_nc():
    nc = bacc.Bacc(None, target_bir_lowering=False, num_swdge_queues=2)
    xs = [nc.dram_tensor(f"x{i+1}", [B_LOC, D], F32, kind="ExternalInput") for i in range(NB)]
    Was = [nc.dram_tensor(f"W_att{i+1}", [D, INT], F32, kind="ExternalInput") for i in range(NB)]
    bas = [nc.dram_tensor(f"b_att{i+1}", [1, INT], F32, kind="ExternalInput") for i in range(NB)]
    Wts = [nc.dram_tensor(f"W_tr{i+1}", [D, OUT], F32, kind="ExternalInput") for i in range(NB)]
    bts = [nc.dram_tensor(f"b_tr{i+1}", [1, OUT], F32, kind="ExternalInput") for i in range(NB)]
    v = nc.dram_tensor("v", [INT, 1], F32, kind="ExternalInput")
    out = nc.dram_tensor("out", [B_LOC, OUT], F32, kind="ExternalOutput")

    with tile.TileContext(nc) as tc:
        with (
            tc.tile_pool(name="wpool", bufs=1) as wpool,
            tc.tile_pool(name="work", bufs=2) as pool,
            tc.tile_pool(name="xfpool", bufs=4) as xfpool,
            tc.tile_pool(name="xbpool", bufs=3) as xbpool,
            tc.tile_pool(name="xtpool", bufs=6) as xtpool,
            tc.tile_pool(name="accpool", bufs=3) as accpool,
            tc.tile_pool(name="psum", bufs=4, space="PSUM") as psum,
            tc.tile_pool(name="ptr", bufs=4, space="PSUM") as ptrpool,
        ):
            xf_q, xb_q, xT_q = {}, {}, {}

            def issue_load(t):
                # HWDGE plain fp32 loads on the SP ring (never SWDGE: SWDGE
                # DMAs serialize against DMA-transposes in hardware; and not
                # the ACT ring, so sigmoids are never queued behind load waits)
                xf = xfpool.tile([128, NB, D], F32, tag="xf")
                for i in range(NB):
                    nc.sync.dma_start(out=xf[:, i, :], in_=xs[i][t * 128:(t + 1) * 128, :])
                xf_q[t] = xf

            def issue_cast(t):
                xb = xbpool.tile([128, NB, D], BF16, tag="xb")
                xf = xf_q.pop(t)
                nc.vector.tensor_copy(xb.rearrange("p a b -> p (a b)"),
                                      xf.rearrange("p a b -> p (a b)"))
                xb_q[t] = xb

            def issue_transpose(t):
                xT = xtpool.tile([128, NB, KC, 128], BF16, tag="xT")
                xbt = xb_q.pop(t)
                for i in range(NB):
                    nc.sync.dma_start(out=xT[:, i, :, :], in_=xbt[:, i, :], transpose=True)
                xT_q[t] = xT

            # ---- setup. Order matters: tiny bias/v loads + broadcasts first
            # (done by ~8us), then the bulky SWDGE weight loads (must complete
            # before any transpose issues: SWDGE serializes vs transposes),
            # with the x fp32 loads racing on the SP HWDGE queue. ----
            ones128 = wpool.tile([128, 128], BF16, tag="ones128")
            nc.vector.memset(ones128[:], 1.0)
            ones32 = wpool.tile([1, 128], F32, tag="ones32")
            nc.vector.memset(ones32[:], 1.0)

            ba_rep, bt_rep = [], []
            for i in range(NB):
                bar = wpool.tile([1, INT], F32, tag=f"bar{i}")
                nc.scalar.dma_start(out=bar[:], in_=bas[i][:])
                p_b = psum.tile([128, INT], F32, tag="att")
                nc.tensor.matmul(p_b[:], lhsT=ones32[:], rhs=bar[:], start=True, stop=True)
                barep = wpool.tile([128, INT], BF16, tag=f"barep{i}")
                nc.scalar.activation(barep[:], p_b[:], AF.Copy, scale=1.0 / 128.0)
                ba_rep.append(barep)
                btr = wpool.tile([1, OUT], F32, tag=f"btr{i}")
                nc.scalar.dma_start(out=btr[:], in_=bts[i][:])
                p_c = ptrpool.tile([128, OUT], F32, tag="tr")
                nc.tensor.matmul(p_c[:], lhsT=ones32[:], rhs=btr[:], start=True, stop=True)
                btrep = wpool.tile([128, OUT], BF16, tag=f"btrep{i}")
                nc.scalar.activation(btrep[:], p_c[:], AF.Copy, scale=1.0 / 128.0)
                bt_rep.append(btrep)

            v_row = wpool.tile([1, INT], F32, tag="vrow")
            nc.scalar.dma_start(out=v_row[:], in_=v.rearrange("a b -> b a"))
            p_v = psum.tile([128, INT], F32, tag="att")
            nc.tensor.matmul(p_v[:], lhsT=ones32[:], rhs=v_row[:], start=True, stop=True)
            v2_rep = wpool.tile([128, INT], F32, tag="v2rep")
            nc.scalar.activation(v2_rep[:], p_v[:], AF.Copy, scale=2.0)
            vsum = wpool.tile([128, 1], F32, tag="vsum")
            nc.vector.reduce_sum(vsum[:], p_v[:], axis=mybir.AxisListType.X)
            neg_vsum = wpool.tile([128, 1], F32, tag="nvsum")
            nc.vector.tensor_scalar_mul(neg_vsum[:], vsum[:], -1.0)

            issue_load(0)
            Wa_sb, Wt_sb = [], []
            for i in range(NB):
                wa = wpool.tile([128, KC, INT], BF16, tag=f"wa{i}")
                nc.gpsimd.dma_start(out=wa[:], in_=Was[i].rearrange("(c p) n -> p c n", p=128))
                Wa_sb.append(wa)
            issue_load(1)
            for i in range(NB):
                wt = wpool.tile([128, KC, OUT], BF16, tag=f"wt{i}")
                nc.gpsimd.dma_start(out=wt[:], in_=Wts[i].rearrange("(c p) n -> p c n", p=128))
                Wt_sb.append(wt)
            issue_load(2)
            issue_cast(0)
            issue_cast(1)
            issue_transpose(0)
            issue_load(3)
            issue_cast(2)
            issue_transpose(1)

            # ---- main loop ----
            for t in range(N_TILES):
                if t + 2 < N_TILES:
                    issue_transpose(t + 2)
                if t + 4 < N_TILES:
                    issue_load(t + 4)
                if t + 3 < N_TILES:
                    issue_cast(t + 3)
                xT = xT_q.pop(t)

                p_atts = []
                for i in range(NB):
                    p_att = psum.tile([128, INT], F32, tag="att")
                    for c in range(KC):
                        nc.tensor.matmul(p_att[:], lhsT=xT[:, i, c, :], rhs=Wa_sb[i][:, c, :],
                                         start=(c == 0), stop=False)
                    nc.tensor.matmul(p_att[:], lhsT=ones128[:], rhs=ba_rep[i][:], start=False, stop=True)
                    p_atts.append(p_att)
                p_trs = []
                for i in range(NB):
                    p_tr = ptrpool.tile([128, OUT], F32, tag="tr")
                    for c in range(KC):
                        nc.tensor.matmul(p_tr[:], lhsT=xT[:, i, c, :], rhs=Wt_sb[i][:, c, :],
                                         start=(c == 0), stop=False)
                    nc.tensor.matmul(p_tr[:], lhsT=ones128[:], rhs=bt_rep[i][:], start=False, stop=True)
                    p_trs.append(p_tr)

                l3 = pool.tile([128, 4], F32, tag="l3")
                for i in range(NB):
                    sgh = pool.tile([128, INT], F32, tag=f"sgh{i}")
                    nc.scalar.activation(sgh[:], p_atts[i][:], AF.Sigmoid, scale=2.0)
                    prod = pool.tile([128, INT], F32, tag=f"prod{i}")
                    nc.vector.scalar_tensor_tensor(prod[:], sgh[:], 1.0, v2_rep[:],
                                                   OP.mult, OP.mult, accum_out=l3[:, i:i + 1])

                sg3 = pool.tile([128, 4], F32, tag="sg3")
                nc.scalar.activation(sg3[:, 0:NB], l3[:, 0:NB], AF.Sigmoid, bias=neg_vsum[:])
                u3 = pool.tile([128, 4], F32, tag="u3")
                nc.vector.tensor_scalar(u3[:, 0:NB], sg3[:, 0:NB], -1.0, 1.0, OP.mult, OP.add)
                w3 = pool.tile([128, 4], F32, tag="w3")
                nc.vector.reciprocal(w3[:, 0:NB], u3[:, 0:NB])
                r3 = pool.tile([128, 4], F32, tag="r3")
                ssum = pool.tile([128, 1], F32, tag="ssum")
                nc.vector.scalar_tensor_tensor(r3[:, 0:NB], sg3[:, 0:NB], 1.0, w3[:, 0:NB],
                                               OP.mult, OP.mult, accum_out=ssum[:])
                rs = pool.tile([128, 1], F32, tag="rs")
                nc.vector.reciprocal(rs[:], ssum[:])
                s3 = pool.tile([128, 4], F32, tag="s3")
                nc.vector.tensor_scalar_mul(s3[:, 0:NB], r3[:, 0:NB], rs[:])

                acc0 = accpool.tile([128, OUT], F32, tag="acc0")
                nc.vector.tensor_scalar_mul(acc0[:], p_trs[0][:], s3[:, 0:1])
                acc1 = accpool.tile([128, OUT], F32, tag="acc1")
                nc.vector.scalar_tensor_tensor(acc1[:], p_trs[1][:], s3[:, 1:2], acc0[:],
                                               OP.mult, OP.add)
                acc2 = accpool.tile([128, OUT], F32, tag="acc2")
                nc.vector.scalar_tensor_tensor(acc2[:], p_trs[2][:], s3[:, 2:3], acc1[:],
                                               OP.mult, OP.add)
                # store on the SP ring, issued after this iteration's
                # transposes/loads: its acc2-wait only delays DMAs that are
                # consumed 3+ tiles later; the ACT ring stays DMA-free so the
                # softmax chain is never blocked
                nc.sync.dma_start(out=out[t * 128:(t + 1) * 128, :], in_=acc2[:])
    nc.compile()
    return nc


LAST_RESULTS = None


def kernel(**inputs) -> np.ndarray:
    if "nc" not in _CACHE:
        _CACHE["nc"] = _build_nc()
    nc = _CACHE["nc"]

    shared = {}
    for i in range(NB):
        for k in (f"W_att{i+1}", f"b_att{i+1}", f"W_tr{i+1}", f"b_tr{i+1}"):
            shared[k] = np.ascontiguousarray(np.asarray(inputs[k], dtype=np.float32))
    shared["v"] = np.ascontiguousarray(np.asarray(inputs["v"], dtype=np.float32))

    in_maps = []
    for c in range(NCORES):
        m = dict(shared)
        for i in range(NB):
            m[f"x{i+1}"] = np.ascontiguousarray(
                np.asarray(inputs[f"x{i+1}"], dtype=np.float32)[c * B_LOC:(c + 1) * B_LOC]
            )
        in_maps.append(m)

    res = run_bass_kernel_spmd(nc, in_maps, core_ids=list(range(NCORES)))
    global LAST_RESULTS
    LAST_RESULTS = res
    return np.concatenate([r["out"] for r in res.results], axis=0)


# revision 3
# speedup vs baseline: 2.3040x; 2.2678x over previous
"""Trainium2 Bass kernel for nn_Attention_linearCombination — v7.

out = sum_i softmax_i(tanh(x_i @ W_att_i + b_att_i) @ v) * (x_i @ W_tr_i + b_tr_i)

Data-parallel over batch (16384 -> 8 cores x 2048 rows), weights replicated.
Per core, 16 tiles of 128 rows.

v7 key insight: SWDGE (gpsimd) DMAs and HWDGE DMA-transposes mutually
serialize (~12us per pair, HW-probed) — every cast-load blocked every
transpose in earlier versions. Steady state here uses NO SWDGE at all:
  - x loads: HWDGE plain fp32 on the ACT ring (~340GB/s), cast fp32->bf16
    on DVE (tensor_copy, ~1.9us/tile), per-branch [128,8,128] transposes on
    the SP ring (the 256KB shape runs ~300GB/s; batched shapes drop to 79).
  - weights: SWDGE cast-loads at setup only, completing before the first
    transpose is issued.
  - PE per branch: 8 accumulating bf16 matmuls + uniform K=128 bias matmul
    (ones128 @ b/128 broadcast) for att (N=256) and tr (N=512); uniform
    tile_size keeps att groups at the ~109ns/MM issue floor.
  - att post: ACT Sigmoid reads att PSUM (scale=2); one DVE
    scalar_tensor_tensor gives sgh*2v with accum_out = logit.
  - softmax: sg3 = Sigmoid(raw + bias=-sum(v)); sigma-ratio softmax, row-sum
    via stt accum_out. combine: 3 fused DVE stt ops reading tr PSUM.
  - stores on the ACT ring.
"""
import numpy as np

import concourse.bass as bass
import concourse.bacc as bacc
import concourse.mybir as mybir
import concourse.tile as tile
from concourse.bass_utils import run_bass_kernel_spmd

F32 = mybir.dt.float32
BF16 = mybir.dt.bfloat16
AF = mybir.ActivationFunctionType
OP = mybir.AluOpType

B = 16384
D = 1024
INT = 256
OUT = 512
NB = 3
NCORES = 8
B_LOC = B // NCORES
KC = D // 128
N_TILES = B_LOC // 128

_CACHE = {}


def _build_nc():
    nc = bacc.Bacc(None, target_bir_lowering=False, num_swdge_queues=2)
    xs = [nc.dram_tensor(f"x{i+1}", [B_LOC, D], F32, kind="ExternalInput") for i in range(NB)]
    Was = [nc.dram_tensor(f"W_att{i+1}", [D, INT], F32, kind="ExternalInput") for i in range(NB)]
    bas = [nc.dram_tensor(f"b_att{i+1}", [1, INT], F32, kind="ExternalInput") for i in range(NB)]
    Wts = [nc.dram_tensor(f"W_tr{i+1}", [D, OUT], F32, kind="ExternalInput") for i in range(NB)]
    bts = [nc.dram_tensor(f"b_tr{i+1}", [1, OUT], F32, kind="ExternalInput") for i in range(NB)]
    v = nc.dram_tensor("v", [INT, 1], F32, kind="ExternalInput")
    out = nc.dram_tensor("out", [B_LOC, OUT], F32, kind="ExternalOutput")

    with tile.TileContext(nc) as tc:
        with (
            tc.tile_pool(name="wpool", bufs=1) as wpool,
            tc.tile_pool(name="work", bufs=2) as pool,
            tc.tile_pool(name="xfpool", bufs=5) as xfpool,
            tc.tile_pool(name="xbpool", bufs=3) as xbpool,
            tc.tile_pool(name="xtpool", bufs=6) as xtpool,
            tc.tile_pool(name="accpool", bufs=3) as accpool,
            tc.tile_pool(name="psum", bufs=4, space="PSUM") as psum,
            tc.tile_pool(name="ptr", bufs=4, space="PSUM") as ptrpool,
        ):
            xf_q, xb_q, xT_q = {}, {}, {}

            def issue_load(t):
                # HWDGE plain fp32 loads on the SP ring (never SWDGE: SWDGE
                # DMAs serialize against DMA-transposes in hardware; and not
                # the ACT ring, so sigmoids are never queued behind load waits)
                xf = xfpool.tile([128, NB, D], F32, tag="xf")
                for i in range(NB):
                    nc.sync.dma_start(out=xf[:, i, :], in_=xs[i][t * 128:(t + 1) * 128, :])
                xf_q[t] = xf

            def issue_cast(t):
                xb = xbpool.tile([128, NB, D], BF16, tag="xb")
                xf = xf_q.pop(t)
                nc.vector.tensor_copy(xb.rearrange("p a b -> p (a b)"),
                                      xf.rearrange("p a b -> p (a b)"))
                xb_q[t] = xb

            def issue_transpose(t):
                xT = xtpool.tile([128, NB, KC, 128], BF16, tag="xT")
                xbt = xb_q.pop(t)
                for i in range(NB):
                    nc.sync.dma_start(out=xT[:, i, :, :], in_=xbt[:, i, :], transpose=True)
                xT_q[t] = xT

            # ---- setup. Order matters: tiny bias/v loads + broadcasts first
            # (done by ~8us), then the bulky SWDGE weight loads (must complete
            # before any transpose issues: SWDGE serializes vs transposes),
            # with the x fp32 loads racing on the SP HWDGE queue. ----
            ones128 = wpool.tile([128, 128], BF16, tag="ones128")
            nc.vector.memset(ones128[:], 1.0)
            ones32 = wpool.tile([1, 128], F32, tag="ones32")
            nc.vector.memset(ones32[:], 1.0)

            ba_rep, bt_rep = [], []
            for i in range(NB):
                bar = wpool.tile([1, INT], F32, tag=f"bar{i}")
                nc.scalar.dma_start(out=bar[:], in_=bas[i][:])
                p_b = psum.tile([128, INT], F32, tag="att")
                nc.tensor.matmul(p_b[:], lhsT=ones32[:], rhs=bar[:], start=True, stop=True)
                barep = wpool.tile([128, INT], BF16, tag=f"barep{i}")
                nc.scalar.activation(barep[:], p_b[:], AF.Copy, scale=1.0 / 128.0)
                ba_rep.append(barep)
                btr = wpool.tile([1, OUT], F32, tag=f"btr{i}")
                nc.scalar.dma_start(out=btr[:], in_=bts[i][:])
                p_c = ptrpool.tile([128, OUT], F32, tag="tr")
                nc.tensor.matmul(p_c[:], lhsT=ones32[:], rhs=btr[:], start=True, stop=True)
                btrep = wpool.tile([128, OUT], BF16, tag=f"btrep{i}")
                nc.scalar.activation(btrep[:], p_c[:], AF.Copy, scale=1.0 / 128.0)
                bt_rep.append(btrep)

            v_row = wpool.tile([1, INT], F32, tag="vrow")
            nc.scalar.dma_start(out=v_row[:], in_=v.rearrange("a b -> b a"))
            p_v = psum.tile([128, INT], F32, tag="att")
            nc.tensor.matmul(p_v[:], lhsT=ones32[:], rhs=v_row[:], start=True, stop=True)
            v2_rep = wpool.tile([128, INT], F32, tag="v2rep")
            nc.scalar.activation(v2_rep[:], p_v[:], AF.Copy, scale=2.0)
            vsum = wpool.tile([128, 1], F32, tag="vsum")
            nc.vector.reduce_sum(vsum[:], p_v[:], axis=mybir.AxisListType.X)
            neg_vsum = wpool.tile([128, 1], F32, tag="nvsum")
            nc.vector.tensor_scalar_mul(neg_vsum[:], vsum[:], -1.0)

            issue_load(0)
            Wa_sb, Wt_sb = [], []
            for i in range(NB):
                wa = wpool.tile([128, KC, INT], BF16, tag=f"wa{i}")
                nc.gpsimd.dma_start(out=wa[:], in_=Was[i].rearrange("(c p) n -> p c n", p=128))
                Wa_sb.append(wa)
            issue_load(1)
            for i in range(NB):
                wt = wpool.tile([128, KC, OUT], BF16, tag=f"wt{i}")
                nc.gpsimd.dma_start(out=wt[:], in_=Wts[i].rearrange("(c p) n -> p c n", p=128))
                Wt_sb.append(wt)
            issue_load(2)
            issue_cast(0)
            issue_cast(1)
            issue_transpose(0)
            issue_load(3)
            issue_load(4)
            issue_cast(2)
            issue_transpose(1)

            # ---- main loop ----
            for t in range(N_TILES):
                if t + 2 < N_TILES:
                    issue_transpose(t + 2)
                if t + 5 < N_TILES:
                    issue_load(t + 5)
                if t + 3 < N_TILES:
                    issue_cast(t + 3)
                xT = xT_q.pop(t)

                p_atts = []
                for i in range(NB):
                    p_att = psum.tile([128, INT], F32, tag="att")
                    for c in range(KC):
                        nc.tensor.matmul(p_att[:], lhsT=xT[:, i, c, :], rhs=Wa_sb[i][:, c, :],
                                         start=(c == 0), stop=False)
                    nc.tensor.matmul(p_att[:], lhsT=ones128[:], rhs=ba_rep[i][:], start=False, stop=True)
                    p_atts.append(p_att)
                p_trs = []
                for i in range(NB):
                    p_tr = ptrpool.tile([128, OUT], F32, tag="tr")
                    for c in range(KC):
                        nc.tensor.matmul(p_tr[:], lhsT=xT[:, i, c, :], rhs=Wt_sb[i][:, c, :],
                                         start=(c == 0), stop=False)
                    nc.tensor.matmul(p_tr[:], lhsT=ones128[:], rhs=bt_rep[i][:], start=False, stop=True)
                    p_trs.append(p_tr)

                l3 = pool.tile([128, 4], F32, tag="l3")
                for i in range(NB):
                    sgh = pool.tile([128, INT], F32, tag=f"sgh{i}")
                    nc.scalar.activation(sgh[:], p_atts[i][:], AF.Sigmoid, scale=2.0)
                    prod = pool.tile([128, INT], F32, tag=f"prod{i}")
                    nc.vector.scalar_tensor_tensor(prod[:], sgh[:], 1.0, v2_rep[:],
                                                   OP.mult, OP.mult, accum_out=l3[:, i:i + 1])

                sg3 = pool.tile([128, 4], F32, tag="sg3")
                nc.scalar.activation(sg3[:, 0:NB], l3[:, 0:NB], AF.Sigmoid, bias=neg_vsum[:])
                u3 = pool.tile([128, 4], F32, tag="u3")
                nc.vector.tensor_scalar(u3[:, 0:NB], sg3[:, 0:NB], -1.0, 1.0, OP.mult, OP.add)
                w3 = pool.tile([128, 4], F32, tag="w3")
                nc.vector.reciprocal(w3[:, 0:NB], u3[:, 0:NB])
                r3 = pool.tile([128, 4], F32, tag="r3")
                ssum = pool.tile([128, 1], F32, tag="ssum")
                nc.vector.scalar_tensor_tensor(r3[:, 0:NB], sg3[:, 0:NB], 1.0, w3[:, 0:NB],
                                               OP.mult, OP.mult, accum_out=ssum[:])
                rs = pool.tile([128, 1], F32, tag="rs")
                nc.vector.reciprocal(rs[:], ssum[:])
                s3 = pool.tile([128, 4], F32, tag="s3")
                nc.vector.tensor_scalar_mul(s3[:, 0:NB], r3[:, 0:NB], rs[:])

                acc0 = accpool.tile([128, OUT], F32, tag="acc0")
                nc.vector.tensor_scalar_mul(acc0[:], p_trs[0][:], s3[:, 0:1])
                acc1 = accpool.tile([128, OUT], F32, tag="acc1")
                nc.vector.scalar_tensor_tensor(acc1[:], p_trs[1][:], s3[:, 1:2], acc0[:],
                                               OP.mult, OP.add)
                acc2 = accpool.tile([128, OUT], F32, tag="acc2")
                nc.vector.scalar_tensor_tensor(acc2[:], p_trs[2][:], s3[:, 2:3], acc1[:],
                                               OP.mult, OP.add)
                # store on the SP ring, issued after this iteration's
                # transposes/loads: its acc2-wait only delays DMAs that are
                # consumed 3+ tiles later; the ACT ring stays DMA-free so the
                # softmax chain is never blocked
                nc.sync.dma_start(out=out[t * 128:(t + 1) * 128, :], in_=acc2[:])
    nc.compile()
    return nc


LAST_RESULTS = None


def kernel(**inputs) -> np.ndarray:
    if "nc" not in _CACHE:
        _CACHE["nc"] = _build_nc()
    nc = _CACHE["nc"]

    shared = {}
    for i in range(NB):
        for k in (f"W_att{i+1}", f"b_att{i+1}", f"W_tr{i+1}", f"b_tr{i+1}"):
            shared[k] = np.ascontiguousarray(np.asarray(inputs[k], dtype=np.float32))
    shared["v"] = np.ascontiguousarray(np.asarray(inputs["v"], dtype=np.float32))

    in_maps = []
    for c in range(NCORES):
        m = dict(shared)
        for i in range(NB):
            m[f"x{i+1}"] = np.ascontiguousarray(
                np.asarray(inputs[f"x{i+1}"], dtype=np.float32)[c * B_LOC:(c + 1) * B_LOC]
            )
        in_maps.append(m)

    res = run_bass_kernel_spmd(nc, in_maps, core_ids=list(range(NCORES)))
    global LAST_RESULTS
    LAST_RESULTS = res
    return np.concatenate([r["out"] for r in res.results], axis=0)
